# revision 48
# baseline (speedup 1.0000x reference)
"""DetConB loss (nn_DetConBLoss) on 8 TRN2 NeuronCores via Bass/Tile.

Strategy (data-parallel over batch, targets replicated):
  - Host: l2-normalize preds/targets in f32, flatten to (4096, 256),
    transpose to (d, rows), cast fp8. Core c owns pred rows
    [c*512, (c+1)*512). Each core receives the full targets with columns
    rolled by c*512 so its own-image diagonal band sits at a fixed,
    compile-time-constant column range (the program is SPMD-identical).
  - Device (per core), per (pred, target) combination u of 4:
    * NORMAL part, target cols [0, CN): per 128-row tile, fp8 DoubleRow
      matmuls into PSUM; ScalarE exp (fused scale) with the ACT
      accumulator producing row sums for free.
    * TRANSPOSED part, target cols [CN, 4096): blocks of 128 target
      cols become the PSUM partition dim (lhsT = target slice, rhs =
      all 512 preds). Each block's exp lands in SBUF as fp8 bit
      patterns: DVE blocks via a Schraudolph uint8 construct
      (y = x*SA8 + SB8, bitcast fp8e4), ACT blocks via exp with fp8e4
      output. A ones-vector DoubleRow matmul then contracts each block
      pair over its 256 target partitions, accumulating per-pred
      denominator partials in PSUM — the reduction runs on the
      TensorE, freeing both vector engines.
    Only ~40 KB of row-sum partials leave the device.
  - Host: the 16x16 own-image diagonal dot blocks (recomputed from the
    same fp8 inputs), masks from the roi indices, positive-pair sums,
    the -inf masking correction, log, and the final mean.
"""
import numpy as np
import ml_dtypes

import concourse.bacc as bacc
import concourse.mybir as mybir
import concourse.tile as tile
from concourse.bass_utils import run_bass_kernel_spmd

TEMP = 0.1
EPS = 1e-11
SCALE = float(np.float32(1.0 / (TEMP + EPS)))
NCORES = 8
B, N, D = 256, 16, 256
R = B * N          # 4096 flat rows
RPC = R // NCORES  # 512 rows per core
MT = RPC // 128    # 4 row-tiles of 128 per core
BF16 = mybir.dt.bfloat16
FP8 = mybir.dt.float8e4
U8 = mybir.dt.uint8
NPFP8 = ml_dtypes.float8_e4m3
F32 = mybir.dt.float32
I32 = mybir.dt.int32

NPAIR = R // 256             # 16 transposed 256-target-column pairs per combo


def is_act_pair(px, pair):
    """Consumer engine for (combo px, pair): alternates so each interleaved
    slot pair has one ScalarE and one DVE consumer."""
    return pair % 2 == 1

# Schraudolph fast-exp, f32 flavor (normal part no longer uses it; kept
# for reference/testing) and e4m3 flavor for the transposed DVE blocks:
# exp(x) ~= bitcast_fp8e4(uint8(x*SA8 + SB8)), x = scale*logit.
SA8 = float(np.float32(8.0 / np.log(2.0)))
SB8 = float(np.float32(7 * 8 - 486411.0 / 2**20))


def build_nc():
    """Build + schedule + compile the SPMD per-core Bass program."""
    nc = bacc.Bacc("TRN2", target_bir_lowering=False, debug=False,
                   num_devices=NCORES)

    # p layout [128, 2048]: k0 at cols [0,512), k1 at [1024,1536), rest pad
    # (the 512-byte gap keeps the DoubleRow k-pair fetch off a single SBUF
    # line; adjacent k-runs halve the PE's fp8 stream rate).
    p_dram = [nc.dram_tensor(f"p{i + 1}t", [128, 4 * RPC], FP8,
                             kind="ExternalInput") for i in range(2)]
    t_dram = [nc.dram_tensor(f"t{i + 1}t", [D, R], FP8, kind="ExternalInput")
              for i in range(2)]
    dsum = nc.dram_tensor("dsum", [4, RPC], F32, kind="ExternalOutput")

    with tile.TileContext(nc) as tc:
        with (
            tc.tile_pool(name="const", bufs=1) as const_pool,
            tc.tile_pool(name="psum", bufs=1, space="PSUM") as psum_pool,
            tc.tile_pool(name="scratch", bufs=2) as scratch_pool,
        ):
            t_sb = [const_pool.tile([128, 2 * R], FP8, name=f"t_sb{i}", tag=f"t{i}")
                    for i in range(2)]
            p_sb = [const_pool.tile([128, 4 * RPC], FP8, name=f"p_sb{i}", tag=f"p{i}")
                    for i in range(2)]

            warm = const_pool.tile([128, 2], F32, name="warm", tag="warm")
            zbias = const_pool.tile([128, 1], F32, name="zbias", tag="zbias")
            ones8 = const_pool.tile([128, 64], FP8, name="ones8", tag="ones8")

            # Input DMAs. The three loads gating the first matmul (both k
            # halves of t1's first 512 cols + p1) are spread across THREE
            # descriptor-generation engines — gpsimd (SWDGE), Activation,
            # and sync — and emitted before everything else so each engine's
            # earliest cycles go to them.
            def load_t_cols(tsel, k, c0, c1, eng=None):
                (eng or nc.sync).dma_start(
                    out=t_sb[tsel][:, k * R + c0: k * R + c1],
                    in_=t_dram[tsel][k * 128:(k + 1) * 128, c0:c1])

            def load_t(tsel, k, g):
                load_t_cols(tsel, k, g * 2048, (g + 1) * 2048)

            def load_p(px, eng):
                # only the two used 512-col halves, not the padding
                for k in range(2):
                    eng.dma_start(
                        out=p_sb[px][:, k * 2 * RPC:k * 2 * RPC + RPC],
                        in_=p_dram[px].ap()[:, k * 2 * RPC:k * 2 * RPC + RPC])

            load_t_cols(0, 0, 0, 512, eng=nc.gpsimd)
            load_t_cols(0, 1, 0, 512, eng=nc.scalar)
            load_p(0, nc.sync)
            load_p(1, nc.scalar)
            load_t_cols(0, 0, 512, 2048)
            load_t_cols(0, 1, 512, 2048)

            nc.vector.memset(warm, 0.0)
            # Explicit zero-bias AP: a float bias would be lowered through the
            # const-AP machinery, whose TENSOR_LOAD sits in the preamble.
            nc.vector.memset(zbias, 0.0)
            # 32 identical ones columns: a DoubleRow LDWEIGHTS with a single
            # weight column fails the compiler's ISA check, so the sum
            # matmul produces 32 duplicate rows (only row 0 is read).
            nc.vector.memset(ones8, 1.0)
            # Warm the exp table set during the input-DMA window so the first
            # real ACTIVATE does not pay the ~2.7us ACT_TABLE_LOAD.
            nc.scalar.activation(warm, warm,
                                 mybir.ActivationFunctionType.Exp, bias=zbias)

            rhs3 = [t_sb[i].rearrange("p (k c) -> p k c", k=2) for i in range(2)]
            lhs3 = [p_sb[i].rearrange("p (k c) -> p k c", k=2) for i in range(2)]
            onesT = ones8.rearrange("p (k m) -> p k m", k=2)

            # Per-pred denominator partials: combo u accumulates into
            # partitions [0,32), column half u%2, of one persistent tile
            # (combo u+2 reuses the half after u's row is copied out).
            psum_acc = psum_pool.tile([128, 2 * RPC], F32, name="psum_acc",
                                      tag="pss")
            dsb = const_pool.tile([128, RPC], F32, name="dsb", tag="dsb")

            def mm2(px, pair, sch):
                # ones-DoubleRow matmul: contract the pair's 256 target
                # partitions; accumulate per-pred sums in px's column half.
                nc.tensor.matmul(
                    psum_acc[0:32, px * RPC:(px + 1) * RPC],
                    onesT,
                    sch.bitcast(FP8).rearrange(
                        "p (k c) -> p k c", k=2)[:, :, 0:RPC],
                    start=(pair == 0),
                    stop=(pair == NPAIR - 1),
                    perf_mode=mybir.MatmulPerfMode.DoubleRow)

            # The MM2 pending queue is global: a combo's last sum matmuls
            # flush interleaved into the NEXT combo's matmul stream (the two
            # psum_acc halves don't conflict), so the PE never drains at a
            # combo boundary. Each combo's finished row is bounced to SBUF
            # (PSUM is not DMA-readable) and shipped right after its final
            # sum matmul; the copy also frees the column half for combo+2.
            pending = []

            def flush_one():
                combo, px, pair, sch = pending.pop(0)
                mm2(px, pair, sch)
                if pair == NPAIR - 1:
                    nc.scalar.copy(
                        dsb[combo * 32:combo * 32 + 1, :],
                        psum_acc[0:1, px * RPC:(px + 1) * RPC])
                    nc.sync.dma_start(
                        out=dsum.ap()[combo:combo + 1, :],
                        in_=dsb[combo * 32:combo * 32 + 1, :])

            for tsel in range(2):
                for px in range(2):
                    combo = tsel * 2 + px
                    for pair in range(NPAIR):
                        if combo == 0 and pair == 1:
                            # Same dependency-anchor trick for t1's second
                            # half (1 MB, first needed at pair 8): delay its
                            # transfers past the early pairs, in 512-col
                            # chunks so completion sems land just in time.
                            for c0 in (2048, 3072):
                                for k in range(2):
                                    nc.vector.memset(
                                        t_sb[0][:, k * R + c0:
                                                k * R + c0 + 1], 0.0)
                            for c0 in (2048, 3072):
                                for k in range(2):
                                    load_t_cols(0, k, c0, c0 + 1024)
                        if combo == 0 and pair == 8:
                            # t2 is first needed at combo 2 (~half-way); a
                            # 1-col memset anchor on the DVE queue delays its
                            # 1 MB of DMA traffic past the first combo, whose
                            # matmuls otherwise lose fp8 fetch bandwidth to
                            # the incoming stream (Q0 MMs 486ns vs 380).
                            for k in range(2):
                                for g in range(2):
                                    nc.vector.memset(
                                        t_sb[1][:, k * R + g * 2048:
                                                k * R + g * 2048 + 1], 0.0)
                            for k in range(2):
                                for g in range(2):
                                    load_t(1, k, g)
                        # pair = 256 target cols as PSUM partitions: two
                        # 128-col DoubleRow matmuls into one 2-bank tile.
                        psT = psum_pool.tile([128, 2 * RPC], F32,
                                             name="psT", tag="pst", bufs=3)
                        for half in range(2):
                            tc0 = (2 * pair + half) * 128
                            nc.tensor.matmul(
                                psT[:, half * RPC:(half + 1) * RPC],
                                rhs3[tsel][:, :, tc0:tc0 + 128],
                                lhs3[px][:, :, 0:RPC],
                                start=True, stop=True,
                                perf_mode=mybir.MatmulPerfMode.DoubleRow)
                        # The pair's sum matmul is emitted three pairs later
                        # so the in-order PE queue never head-blocks waiting
                        # for the exp data.
                        if len(pending) == 3:
                            flush_one()
                        # One fused consumer per pair turns both halves into
                        # fp8e4 exp bit patterns in the gapped sch slots
                        # (k-runs 1 KB apart keep the PE fetch at full rate).
                        sch = scratch_pool.tile([128, 2048], U8,
                                                name="sch", tag="sch", bufs=6)
                        pin = psT.rearrange("p (k c) -> p k c", k=2)
                        pout = sch.rearrange("p (k c) -> p k c", k=2)[:, :, 0:RPC]
                        if is_act_pair(px, pair):
                            nc.scalar.activation(
                                pout.bitcast(FP8), pin,
                                mybir.ActivationFunctionType.Exp,
                                bias=zbias, scale=SCALE)
                        else:
                            nc.vector.tensor_scalar(
                                pout, pin, SA8 * SCALE, SB8,
                                op0=mybir.AluOpType.mult,
                                op1=mybir.AluOpType.add)
                        pending.append((combo, px, pair, sch))
            while pending:
                flush_one()

    nc.compile()
    return nc


_NC = None


def _get_nc():
    global _NC
    if _NC is None:
        _NC = build_nc()
    return _NC


def _l2norm(x):
    return x / np.linalg.norm(x, axis=-1, keepdims=True)


def _dev_p_layout(pt):
    # pt: [D=256, RPC] fp8 -> [128, 2048] with k0 at [0,512), k1 at [1024,1536)
    out = np.zeros((128, 4 * RPC), NPFP8)
    out[:, 0:RPC] = pt[0:128]
    out[:, 2 * RPC:3 * RPC] = pt[128:256]
    return out


def host_prep(pred1, pred2, target1, target2):
    p1t = _l2norm(np.asarray(pred1, np.float32)).reshape(R, D).T.astype(NPFP8)
    p2t = _l2norm(np.asarray(pred2, np.float32)).reshape(R, D).T.astype(NPFP8)
    t1t = _l2norm(np.asarray(target1, np.float32)).reshape(R, D).T.astype(NPFP8)
    t2t = _l2norm(np.asarray(target2, np.float32)).reshape(R, D).T.astype(NPFP8)
    # Raw own-image diagonal dot blocks (b, n, m), fp8-quantized operands in
    # f32 — the same products the device computes, ~0.4% of total FLOPs.
    pf = [p1t.T.astype(np.float32).reshape(B, N, D),
          p2t.T.astype(np.float32).reshape(B, N, D)]
    tf = [t1t.T.astype(np.float32).reshape(B, N, D),
          t2t.T.astype(np.float32).reshape(B, N, D)]
    diag = [[np.einsum('bnd,bmd->bnm', pf[px], tf[ts]).astype(np.float32)
             for ts in range(2)] for px in range(2)]
    in_maps = []
    for c in range(NCORES):
        r0 = c * RPC
        in_maps.append({
            "p1t": _dev_p_layout(p1t[:, r0:r0 + RPC]),
            "p2t": _dev_p_layout(p2t[:, r0:r0 + RPC]),
            "t1t": np.ascontiguousarray(np.concatenate([t1t[:, r0:], t1t[:, :r0]], axis=1)),
            "t2t": np.ascontiguousarray(np.concatenate([t2t[:, r0:], t2t[:, :r0]], axis=1)),
        })
    return in_maps, diag


def host_post(results, diag, pind1, pind2, tind1, tind2):
    S = np.zeros((2, R), np.float64)
    for c, res in enumerate(results):
        dsumv = np.asarray(res["dsum"]).astype(np.float64)
        for px in range(2):
            r0 = c * RPC
            S[px, r0:r0 + RPC] = dsumv[0 * 2 + px] + dsumv[1 * 2 + px]
    sc = np.float32(SCALE)
    D_aa = sc * diag[0][0]
    D_ab = sc * diag[0][1]
    D_ba = sc * diag[1][0]
    D_bb = sc * diag[1][1]

    f32 = np.float32
    pind1, pind2 = np.asarray(pind1), np.asarray(pind2)
    tind1, tind2 = np.asarray(tind1), np.asarray(tind2)
    same_aa = (pind1[:, :, None] == tind1[:, None, :]).astype(f32)
    same_ab = (pind1[:, :, None] == tind2[:, None, :]).astype(f32)
    same_ba = (pind2[:, :, None] == tind1[:, None, :]).astype(f32)
    same_bb = (pind2[:, :, None] == tind2[:, None, :]).astype(f32)

    S0 = S[0].reshape(B, N)
    S1 = S[1].reshape(B, N)
    corr0 = (same_aa * np.exp(D_aa.astype(np.float64))).sum(-1)
    corr1 = (same_bb * np.exp(D_bb.astype(np.float64))).sum(-1)
    lse0 = np.log(S0 - corr0)
    lse1 = np.log(S1 - corr1)

    num_pos0 = same_ab.sum(-1)
    num_pos1 = same_ba.sum(-1)
    pos_sum0 = (same_ab * D_ab).sum(-1)
    pos_sum1 = (same_ba * D_ba).sum(-1)

    area0 = (pind1[:, :, None] == pind1[:, None, :]).astype(f32).sum(-1)
    area1 = (pind2[:, :, None] == pind2[:, None, :]).astype(f32).sum(-1)
    w0 = (num_pos0 > 0.001).astype(f32) / area0
    w1 = (num_pos1 > 0.001).astype(f32) / area1

    ce0 = -w0 * (pos_sum0 - num_pos0 * lse0) / np.maximum(num_pos0, 1.0)
    ce1 = -w1 * (pos_sum1 - num_pos1 * lse1) / np.maximum(num_pos1, 1.0)
    return np.float32(ce0.mean() + ce1.mean())


def run_hw(inputs, trace=False):
    nc = _get_nc()
    in_maps, diag = host_prep(inputs["pred1"], inputs["pred2"],
                              inputs["target1"], inputs["target2"])
    last_err = None
    for attempt in range(3):
        try:
            res = run_bass_kernel_spmd(nc, in_maps,
                                       core_ids=list(range(NCORES)),
                                       trace=trace)
            break
        except Exception as e:  # transient NRT device errors recover on retry
            last_err = e
            import time
            time.sleep(20 * (attempt + 1))
    else:
        raise last_err
    loss = host_post(res.results, diag, inputs["pind1"], inputs["pind2"],
                     inputs["tind1"], inputs["tind2"])
    return loss, res


def kernel(**inputs):
    loss, _ = run_hw(inputs, trace=False)
    return loss


# revision 49
# speedup vs baseline: 1.0368x; 1.0368x over previous
"""DetConB loss (nn_DetConBLoss) on 8 TRN2 NeuronCores via Bass/Tile.

Strategy (data-parallel over batch, targets replicated):
  - Host: l2-normalize preds/targets in f32, flatten to (4096, 256),
    transpose to (d, rows), cast fp8. Core c owns pred rows
    [c*512, (c+1)*512). Each core receives the full targets with columns
    rolled by c*512 so its own-image diagonal band sits at a fixed,
    compile-time-constant column range (the program is SPMD-identical).
  - Device (per core), per (pred, target) combination u of 4:
    * NORMAL part, target cols [0, CN): per 128-row tile, fp8 DoubleRow
      matmuls into PSUM; ScalarE exp (fused scale) with the ACT
      accumulator producing row sums for free.
    * TRANSPOSED part, target cols [CN, 4096): blocks of 128 target
      cols become the PSUM partition dim (lhsT = target slice, rhs =
      all 512 preds). Each block's exp lands in SBUF as fp8 bit
      patterns: DVE blocks via a Schraudolph uint8 construct
      (y = x*SA8 + SB8, bitcast fp8e4), ACT blocks via exp with fp8e4
      output. A ones-vector DoubleRow matmul then contracts each block
      pair over its 256 target partitions, accumulating per-pred
      denominator partials in PSUM — the reduction runs on the
      TensorE, freeing both vector engines.
    Only ~40 KB of row-sum partials leave the device.
  - Host: the 16x16 own-image diagonal dot blocks (recomputed from the
    same fp8 inputs), masks from the roi indices, positive-pair sums,
    the -inf masking correction, log, and the final mean.
"""
import numpy as np
import ml_dtypes

import concourse.bacc as bacc
import concourse.mybir as mybir
import concourse.tile as tile
from concourse.bass_utils import run_bass_kernel_spmd

TEMP = 0.1
EPS = 1e-11
SCALE = float(np.float32(1.0 / (TEMP + EPS)))
NCORES = 8
B, N, D = 256, 16, 256
R = B * N          # 4096 flat rows
RPC = R // NCORES  # 512 rows per core
MT = RPC // 128    # 4 row-tiles of 128 per core
BF16 = mybir.dt.bfloat16
FP8 = mybir.dt.float8e4
U8 = mybir.dt.uint8
NPFP8 = ml_dtypes.float8_e4m3
F32 = mybir.dt.float32
I32 = mybir.dt.int32

NPAIR = R // 256             # 16 transposed 256-target-column pairs per combo


def is_act_pair(px, pair):
    """Consumer engine for (combo px, pair): alternates so each interleaved
    slot pair has one ScalarE and one DVE consumer."""
    return pair % 2 == 1

# Schraudolph fast-exp, f32 flavor (normal part no longer uses it; kept
# for reference/testing) and e4m3 flavor for the transposed DVE blocks:
# exp(x) ~= bitcast_fp8e4(uint8(x*SA8 + SB8)), x = scale*logit.
SA8 = float(np.float32(8.0 / np.log(2.0)))
SB8 = float(np.float32(7 * 8 - 486411.0 / 2**20))


def build_nc():
    """Build + schedule + compile the SPMD per-core Bass program."""
    nc = bacc.Bacc("TRN2", target_bir_lowering=False, debug=False,
                   num_devices=NCORES)

    # p layout [128, 2048]: k0 at cols [0,512), k1 at [1024,1536), rest pad
    # (the 512-byte gap keeps the DoubleRow k-pair fetch off a single SBUF
    # line; adjacent k-runs halve the PE's fp8 stream rate).
    p_dram = [nc.dram_tensor(f"p{i + 1}t", [128, 4 * RPC], FP8,
                             kind="ExternalInput") for i in range(2)]
    t_dram = [nc.dram_tensor(f"t{i + 1}t", [D, R], FP8, kind="ExternalInput")
              for i in range(2)]
    dsum = nc.dram_tensor("dsum", [4, RPC], F32, kind="ExternalOutput")

    with tile.TileContext(nc) as tc:
        with (
            tc.tile_pool(name="const", bufs=1) as const_pool,
            tc.tile_pool(name="psum", bufs=1, space="PSUM") as psum_pool,
            tc.tile_pool(name="scratch", bufs=2) as scratch_pool,
        ):
            t_sb = [const_pool.tile([128, 2 * R], FP8, name=f"t_sb{i}", tag=f"t{i}")
                    for i in range(2)]
            p_sb = [const_pool.tile([128, 4 * RPC], FP8, name=f"p_sb{i}", tag=f"p{i}")
                    for i in range(2)]

            warm = const_pool.tile([128, 2], F32, name="warm", tag="warm")
            zbias = const_pool.tile([128, 1], F32, name="zbias", tag="zbias")
            ones8 = const_pool.tile([128, 64], FP8, name="ones8", tag="ones8")

            # Input DMAs. The three loads gating the first matmul (both k
            # halves of t1's first 512 cols + p1) are spread across THREE
            # descriptor-generation engines — gpsimd (SWDGE), Activation,
            # and sync — and emitted before everything else so each engine's
            # earliest cycles go to them.
            def load_t_cols(tsel, k, c0, c1, eng=None):
                (eng or nc.sync).dma_start(
                    out=t_sb[tsel][:, k * R + c0: k * R + c1],
                    in_=t_dram[tsel][k * 128:(k + 1) * 128, c0:c1])

            def load_t(tsel, k, g):
                load_t_cols(tsel, k, g * 2048, (g + 1) * 2048)

            def load_p(px, eng):
                # only the two used 512-col halves, not the padding
                for k in range(2):
                    eng.dma_start(
                        out=p_sb[px][:, k * 2 * RPC:k * 2 * RPC + RPC],
                        in_=p_dram[px].ap()[:, k * 2 * RPC:k * 2 * RPC + RPC])

            load_t_cols(0, 0, 0, 512, eng=nc.gpsimd)
            load_t_cols(0, 1, 0, 512, eng=nc.scalar)
            load_p(0, nc.sync)
            load_p(1, nc.scalar)
            load_t_cols(0, 0, 512, 2048)
            load_t_cols(0, 1, 512, 2048)
            load_t(0, 0, 1)
            load_t(0, 1, 1)

            nc.vector.memset(warm, 0.0)
            # Explicit zero-bias AP: a float bias would be lowered through the
            # const-AP machinery, whose TENSOR_LOAD sits in the preamble.
            nc.vector.memset(zbias, 0.0)
            # 32 identical ones columns: a DoubleRow LDWEIGHTS with a single
            # weight column fails the compiler's ISA check, so the sum
            # matmul produces 32 duplicate rows (only row 0 is read).
            nc.vector.memset(ones8, 1.0)
            # Warm the exp table set during the input-DMA window so the first
            # real ACTIVATE does not pay the ~2.7us ACT_TABLE_LOAD.
            nc.scalar.activation(warm, warm,
                                 mybir.ActivationFunctionType.Exp, bias=zbias)

            rhs3 = [t_sb[i].rearrange("p (k c) -> p k c", k=2) for i in range(2)]
            lhs3 = [p_sb[i].rearrange("p (k c) -> p k c", k=2) for i in range(2)]
            onesT = ones8.rearrange("p (k m) -> p k m", k=2)

            # Per-pred denominator partials: combo u accumulates into
            # partitions [0,32), column half u%2, of one persistent tile
            # (combo u+2 reuses the half after u's row is copied out).
            psum_acc = psum_pool.tile([128, 2 * RPC], F32, name="psum_acc",
                                      tag="pss")
            dsb = const_pool.tile([128, RPC], F32, name="dsb", tag="dsb")

            def mm2(px, pair, sch):
                # ones-DoubleRow matmul: contract the pair's 256 target
                # partitions; accumulate per-pred sums in px's column half.
                nc.tensor.matmul(
                    psum_acc[0:32, px * RPC:(px + 1) * RPC],
                    onesT,
                    sch.bitcast(FP8).rearrange(
                        "p (k c) -> p k c", k=2)[:, :, 0:RPC],
                    start=(pair == 0),
                    stop=(pair == NPAIR - 1),
                    perf_mode=mybir.MatmulPerfMode.DoubleRow)

            # The MM2 pending queue is global: a combo's last sum matmuls
            # flush interleaved into the NEXT combo's matmul stream (the two
            # psum_acc halves don't conflict), so the PE never drains at a
            # combo boundary. Each combo's finished row is bounced to SBUF
            # (PSUM is not DMA-readable) and shipped right after its final
            # sum matmul; the copy also frees the column half for combo+2.
            pending = []

            def flush_one():
                combo, px, pair, sch = pending.pop(0)
                mm2(px, pair, sch)
                if pair == NPAIR - 1:
                    nc.scalar.copy(
                        dsb[combo * 32:combo * 32 + 1, :],
                        psum_acc[0:1, px * RPC:(px + 1) * RPC])
                    nc.sync.dma_start(
                        out=dsum.ap()[combo:combo + 1, :],
                        in_=dsb[combo * 32:combo * 32 + 1, :])

            for tsel in range(2):
                for px in range(2):
                    combo = tsel * 2 + px
                    for pair in range(NPAIR):
                        if combo == 0 and pair == 8:
                            # t2 is first needed at combo 2 (~half-way); a
                            # 1-col memset anchor on the DVE queue delays its
                            # 1 MB of DMA traffic past the first combo, whose
                            # matmuls otherwise lose fp8 fetch bandwidth to
                            # the incoming stream (Q0 MMs 486ns vs 380).
                            for k in range(2):
                                for g in range(2):
                                    nc.vector.memset(
                                        t_sb[1][:, k * R + g * 2048:
                                                k * R + g * 2048 + 1], 0.0)
                            for k in range(2):
                                for g in range(2):
                                    load_t(1, k, g)
                        # pair = 256 target cols as PSUM partitions: two
                        # 128-col DoubleRow matmuls into one 2-bank tile.
                        psT = psum_pool.tile([128, 2 * RPC], F32,
                                             name="psT", tag="pst", bufs=3)
                        for half in range(2):
                            tc0 = (2 * pair + half) * 128
                            nc.tensor.matmul(
                                psT[:, half * RPC:(half + 1) * RPC],
                                rhs3[tsel][:, :, tc0:tc0 + 128],
                                lhs3[px][:, :, 0:RPC],
                                start=True, stop=True,
                                perf_mode=mybir.MatmulPerfMode.DoubleRow)
                        # The pair's sum matmul is emitted three pairs later
                        # so the in-order PE queue never head-blocks waiting
                        # for the exp data.
                        if len(pending) == 3:
                            flush_one()
                        # One fused consumer per pair turns both halves into
                        # fp8e4 exp bit patterns in the gapped sch slots
                        # (k-runs 1 KB apart keep the PE fetch at full rate).
                        sch = scratch_pool.tile([128, 2048], U8,
                                                name="sch", tag="sch", bufs=6)
                        pin = psT.rearrange("p (k c) -> p k c", k=2)
                        pout = sch.rearrange("p (k c) -> p k c", k=2)[:, :, 0:RPC]
                        if is_act_pair(px, pair):
                            nc.scalar.activation(
                                pout.bitcast(FP8), pin,
                                mybir.ActivationFunctionType.Exp,
                                bias=zbias, scale=SCALE)
                        else:
                            nc.vector.tensor_scalar(
                                pout, pin, SA8 * SCALE, SB8,
                                op0=mybir.AluOpType.mult,
                                op1=mybir.AluOpType.add)
                        pending.append((combo, px, pair, sch))
            while pending:
                flush_one()

    nc.compile()
    return nc


_NC = None


def _get_nc():
    global _NC
    if _NC is None:
        _NC = build_nc()
    return _NC


def _l2norm(x):
    return x / np.linalg.norm(x, axis=-1, keepdims=True)


def _dev_p_layout(pt):
    # pt: [D=256, RPC] fp8 -> [128, 2048] with k0 at [0,512), k1 at [1024,1536)
    out = np.zeros((128, 4 * RPC), NPFP8)
    out[:, 0:RPC] = pt[0:128]
    out[:, 2 * RPC:3 * RPC] = pt[128:256]
    return out


def host_prep(pred1, pred2, target1, target2):
    p1t = _l2norm(np.asarray(pred1, np.float32)).reshape(R, D).T.astype(NPFP8)
    p2t = _l2norm(np.asarray(pred2, np.float32)).reshape(R, D).T.astype(NPFP8)
    t1t = _l2norm(np.asarray(target1, np.float32)).reshape(R, D).T.astype(NPFP8)
    t2t = _l2norm(np.asarray(target2, np.float32)).reshape(R, D).T.astype(NPFP8)
    # Raw own-image diagonal dot blocks (b, n, m), fp8-quantized operands in
    # f32 — the same products the device computes, ~0.4% of total FLOPs.
    pf = [p1t.T.astype(np.float32).reshape(B, N, D),
          p2t.T.astype(np.float32).reshape(B, N, D)]
    tf = [t1t.T.astype(np.float32).reshape(B, N, D),
          t2t.T.astype(np.float32).reshape(B, N, D)]
    diag = [[np.einsum('bnd,bmd->bnm', pf[px], tf[ts]).astype(np.float32)
             for ts in range(2)] for px in range(2)]
    in_maps = []
    for c in range(NCORES):
        r0 = c * RPC
        in_maps.append({
            "p1t": _dev_p_layout(p1t[:, r0:r0 + RPC]),
            "p2t": _dev_p_layout(p2t[:, r0:r0 + RPC]),
            "t1t": np.ascontiguousarray(np.concatenate([t1t[:, r0:], t1t[:, :r0]], axis=1)),
            "t2t": np.ascontiguousarray(np.concatenate([t2t[:, r0:], t2t[:, :r0]], axis=1)),
        })
    return in_maps, diag


def host_post(results, diag, pind1, pind2, tind1, tind2):
    S = np.zeros((2, R), np.float64)
    for c, res in enumerate(results):
        dsumv = np.asarray(res["dsum"]).astype(np.float64)
        for px in range(2):
            r0 = c * RPC
            S[px, r0:r0 + RPC] = dsumv[0 * 2 + px] + dsumv[1 * 2 + px]
    sc = np.float32(SCALE)
    D_aa = sc * diag[0][0]
    D_ab = sc * diag[0][1]
    D_ba = sc * diag[1][0]
    D_bb = sc * diag[1][1]

    f32 = np.float32
    pind1, pind2 = np.asarray(pind1), np.asarray(pind2)
    tind1, tind2 = np.asarray(tind1), np.asarray(tind2)
    same_aa = (pind1[:, :, None] == tind1[:, None, :]).astype(f32)
    same_ab = (pind1[:, :, None] == tind2[:, None, :]).astype(f32)
    same_ba = (pind2[:, :, None] == tind1[:, None, :]).astype(f32)
    same_bb = (pind2[:, :, None] == tind2[:, None, :]).astype(f32)

    S0 = S[0].reshape(B, N)
    S1 = S[1].reshape(B, N)
    corr0 = (same_aa * np.exp(D_aa.astype(np.float64))).sum(-1)
    corr1 = (same_bb * np.exp(D_bb.astype(np.float64))).sum(-1)
    lse0 = np.log(S0 - corr0)
    lse1 = np.log(S1 - corr1)

    num_pos0 = same_ab.sum(-1)
    num_pos1 = same_ba.sum(-1)
    pos_sum0 = (same_ab * D_ab).sum(-1)
    pos_sum1 = (same_ba * D_ba).sum(-1)

    area0 = (pind1[:, :, None] == pind1[:, None, :]).astype(f32).sum(-1)
    area1 = (pind2[:, :, None] == pind2[:, None, :]).astype(f32).sum(-1)
    w0 = (num_pos0 > 0.001).astype(f32) / area0
    w1 = (num_pos1 > 0.001).astype(f32) / area1

    ce0 = -w0 * (pos_sum0 - num_pos0 * lse0) / np.maximum(num_pos0, 1.0)
    ce1 = -w1 * (pos_sum1 - num_pos1 * lse1) / np.maximum(num_pos1, 1.0)
    return np.float32(ce0.mean() + ce1.mean())


def run_hw(inputs, trace=False):
    nc = _get_nc()
    in_maps, diag = host_prep(inputs["pred1"], inputs["pred2"],
                              inputs["target1"], inputs["target2"])
    last_err = None
    for attempt in range(3):
        try:
            res = run_bass_kernel_spmd(nc, in_maps,
                                       core_ids=list(range(NCORES)),
                                       trace=trace)
            break
        except Exception as e:  # transient NRT device errors recover on retry
            last_err = e
            import time
            time.sleep(20 * (attempt + 1))
    else:
        raise last_err
    loss = host_post(res.results, diag, inputs["pind1"], inputs["pind2"],
                     inputs["tind1"], inputs["tind2"])
    return loss, res


def kernel(**inputs):
    loss, _ = run_hw(inputs, trace=False)
    return loss
